# revision 1
# baseline (speedup 1.0000x reference)
"""Compact Bilinear Pooling — batch-sharded across 8 workers.

Shapes hardcoded per spec: x (16, 2048, 14, 14) f32, h1/h2 (2048,) int32 in [0,8192),
s1/s2 (2048,) int32 in {0,1}. Output (16, 8192) f32.

Per the count-sketch identity, cbp = sum_l ifft(fft(sketch1_l) * fft(sketch2_l)).real
followed by sum-pool over locations, signed sqrt, and L2 normalization. Batch is
data-parallel: each of the 8 shards processes B/8 = 2 batch elements independently.
"""
import numpy as np

B, C, H, W = 16, 2048, 14, 14
D = 8192
NSHARDS = 8


def _cbp_shard(x_shard, h1, h2, sg1, sg2):
    b = x_shard.shape[0]
    hw = H * W
    xf = x_shard.reshape(b, C, hw).astype(np.float32)
    out = np.empty((b, D), np.float32)
    for i in range(b):
        y1 = np.zeros((D, hw), np.float32)
        y2 = np.zeros((D, hw), np.float32)
        np.add.at(y1, h1, xf[i] * sg1[:, None])
        np.add.at(y2, h2, xf[i] * sg2[:, None])
        f1 = np.fft.rfft(y1, axis=0)
        f2 = np.fft.rfft(y2, axis=0)
        conv = np.fft.irfft(f1 * f2, n=D, axis=0)
        out[i] = conv.sum(axis=1)
    out = np.sign(out) * np.sqrt(np.abs(out) + 1e-5)
    nrm = np.maximum(np.linalg.norm(out, axis=1, keepdims=True), 1e-12)
    return (out / nrm).astype(np.float32)


def kernel(x, h1, h2, s1, s2):
    x = np.asarray(x, np.float32)
    h1 = np.asarray(h1, np.int64)
    h2 = np.asarray(h2, np.int64)
    sg1 = (2 * np.asarray(s1, np.float32) - 1.0)
    sg2 = (2 * np.asarray(s2, np.float32) - 1.0)
    bpc = B // NSHARDS
    shards = [
        _cbp_shard(x[k * bpc:(k + 1) * bpc], h1, h2, sg1, sg2)
        for k in range(NSHARDS)
    ]
    return np.concatenate(shards, axis=0).astype(np.float32)



# revision 2
# speedup vs baseline: 1.3118x; 1.3118x over previous
"""Compact Bilinear Pooling on 8 NeuronCores — matmul-only four-step FFT.

Batch (16) is sharded 2-per-core over 8 devices. The count-sketch is folded
into stage 1 of a four-step DFT (D = 8192 = 64x128): channels are class-sorted
by (h mod 64) on the host, so stage 1 is a dense per-class matmul with
host-built complex weights. All complex arithmetic uses explicit re/im planes
(the neuron backend supports neither fft nor complex dtypes).

  stage1: A_j[n1,k2,l] = sum_i W_j[n1,i,k2] * xs_j[n1,i,l]
  stage2: P_j[k1,k2,l] = sum_n1 T[k2,k1,n1] * A_j[n1,k2,l]
  product: Z[k1,k2] = sum_l P_1 * P_2
  inverse four-step of Z -> cbp, then signed-sqrt + L2 normalize.
"""
import numpy as np
import jax
import jax.numpy as jnp
from functools import partial

B, C, HW, D = 16, 2048, 196, 8192
N1, N2 = 64, 128
NSHARDS = 8
PAD = 64


def _build_tables(h, s):
    h = np.asarray(h, np.int64)
    sg = (2 * np.asarray(s, np.float64) - 1.0)
    n1 = h % N1
    counts = np.bincount(n1, minlength=N1)
    assert counts.max() <= PAD
    order = np.argsort(n1, kind="stable")  # channels grouped by class
    slot_in_class = np.arange(C) - np.repeat(np.cumsum(counts) - counts, counts)
    perm = np.zeros((N1, PAD), np.int64)
    valid = np.zeros((N1, PAD), np.float64)
    perm[n1[order], slot_in_class] = order
    valid[n1[order], slot_in_class] = 1.0
    k2 = np.arange(N2)[None, None, :]
    n2_tab = (h[perm] // N1)[:, :, None]
    W = (sg[perm] * valid)[:, :, None] * np.exp(-2j * np.pi * n2_tab * k2 / N2)
    return perm, np.ascontiguousarray(W.real, np.float32), np.ascontiguousarray(
        W.imag, np.float32
    )


def _consts():
    k1 = np.arange(N1)[None, :, None]
    n1 = np.arange(N1)[None, None, :]
    k2 = np.arange(N2)[:, None, None]
    T = np.exp(-2j * np.pi * (n1 * k2) / D) * np.exp(-2j * np.pi * (k1 * n1) / N1)
    I64 = np.exp(2j * np.pi * np.arange(N1)[:, None] * np.arange(N1)[None, :] / N1)
    I128 = np.exp(2j * np.pi * np.arange(N2)[:, None] * np.arange(N2)[None, :] / N2)
    tw = np.exp(2j * np.pi * np.arange(N2)[None, :] * np.arange(N1)[:, None] / D)
    f32 = lambda a: np.ascontiguousarray(a, np.float32)
    return (f32(T.real), f32(T.imag), f32(I64.real), f32(I64.imag),
            f32(I128.real), f32(I128.imag), f32(tw.real), f32(tw.imag))


def _shard_fn(xs1, xs2, w1r, w1i, w2r, w2i, Tr, Ti, I64r, I64i, I128r, I128i, twr, twi):
    # xs_j: (b, N1, PAD, HW) f32 class-sorted input
    def stage12(xs, wr, wi):
        Ar = jnp.einsum("rik,bril->brkl", wr, xs)
        Ai = jnp.einsum("rik,bril->brkl", wi, xs)
        Pr = jnp.einsum("knm,bmkl->bnkl", Tr, Ar) - jnp.einsum("knm,bmkl->bnkl", Ti, Ai)
        Pi = jnp.einsum("knm,bmkl->bnkl", Tr, Ai) + jnp.einsum("knm,bmkl->bnkl", Ti, Ar)
        return Pr, Pi

    P1r, P1i = stage12(xs1, w1r, w1i)
    P2r, P2i = stage12(xs2, w2r, w2i)
    Zr = jnp.einsum("bnkl,bnkl->bnk", P1r, P2r) - jnp.einsum("bnkl,bnkl->bnk", P1i, P2i)
    Zi = jnp.einsum("bnkl,bnkl->bnk", P1r, P2i) + jnp.einsum("bnkl,bnkl->bnk", P1i, P2r)
    Br = jnp.einsum("tn,bnk->btk", I64r, Zr) - jnp.einsum("tn,bnk->btk", I64i, Zi)
    Bi = jnp.einsum("tn,bnk->btk", I64r, Zi) + jnp.einsum("tn,bnk->btk", I64i, Zr)
    Cr = Br * twr - Bi * twi
    Ci = Br * twi + Bi * twr
    M = jnp.einsum("bmk,kt->bmt", Cr, I128r) - jnp.einsum("bmk,kt->bmt", Ci, I128i)
    cbp = jnp.transpose(M, (0, 2, 1)).reshape(M.shape[0], D) / D
    cbp = jnp.sign(cbp) * jnp.sqrt(jnp.abs(cbp) + 1e-5)
    nrm = jnp.maximum(
        jnp.sqrt(jnp.sum(cbp * cbp, axis=1, keepdims=True)), 1e-12
    )
    return cbp / nrm


_PMAP = None


def kernel(x, h1, h2, s1, s2):
    global _PMAP
    xf = np.asarray(x, np.float32).reshape(B, C, HW)
    perm1, w1r, w1i = _build_tables(h1, s1)
    perm2, w2r, w2i = _build_tables(h2, s2)
    xs1 = xf[:, perm1.ravel(), :].reshape(B, N1, PAD, HW)
    xs2 = xf[:, perm2.ravel(), :].reshape(B, N1, PAD, HW)
    bpc = B // NSHARDS
    xs1 = xs1.reshape(NSHARDS, bpc, N1, PAD, HW)
    xs2 = xs2.reshape(NSHARDS, bpc, N1, PAD, HW)
    if _PMAP is None:
        _PMAP = jax.pmap(
            _shard_fn, in_axes=(0, 0) + (None,) * 12, devices=jax.devices()[:NSHARDS]
        )
    out = _PMAP(xs1, xs2, w1r, w1i, w2r, w2i, *_consts())
    return np.asarray(out).reshape(B, D).astype(np.float32)


# revision 6
# speedup vs baseline: 1.8125x; 1.3817x over previous
"""Compact Bilinear Pooling on 8 NeuronCores — matmul-only four-step FFT.

Batch (16) is sharded 2-per-core over 8 devices. The count-sketch is folded
into stage 1 of a four-step DFT (D = 8192 = 64x128): channels are class-sorted
by (h mod 64) on the host, so stage 1 is a dense per-class matmul with
host-built complex weights. All complex arithmetic uses explicit re/im planes
(the neuron backend supports neither fft nor complex dtypes).

  stage1: A_j[n1,k2,l] = sum_i W_j[n1,i,k2] * xs_j[n1,i,l]
  stage2: P_j[k1,k2,l] = sum_n1 T[k2,k1,n1] * A_j[n1,k2,l]
  product: Z[k1,k2] = sum_l P_1 * P_2
  inverse four-step of Z -> cbp, then signed-sqrt + L2 normalize.
"""
import numpy as np
import ml_dtypes
import jax
import jax.numpy as jnp
from functools import partial

BF16 = ml_dtypes.bfloat16

B, C, HW, D = 16, 2048, 196, 8192
N1, N2 = 64, 128
NSHARDS = 8
PAD = 64


def _build_tables(h, s):
    h = np.asarray(h, np.int64)
    sg = (2 * np.asarray(s, np.float64) - 1.0)
    n1 = h % N1
    counts = np.bincount(n1, minlength=N1)
    assert counts.max() <= PAD
    order = np.argsort(n1, kind="stable")  # channels grouped by class
    slot_in_class = np.arange(C) - np.repeat(np.cumsum(counts) - counts, counts)
    perm = np.zeros((N1, PAD), np.int64)
    valid = np.zeros((N1, PAD), np.float64)
    perm[n1[order], slot_in_class] = order
    valid[n1[order], slot_in_class] = 1.0
    k2 = np.arange(N2)[None, None, :]
    n2_tab = (h[perm] // N1)[:, :, None]
    W = (sg[perm] * valid)[:, :, None] * np.exp(-2j * np.pi * n2_tab * k2 / N2)
    return perm, np.ascontiguousarray(W.real).astype(BF16), np.ascontiguousarray(
        W.imag
    ).astype(BF16)


def _consts():
    k1 = np.arange(N1)[None, :, None]
    n1 = np.arange(N1)[None, None, :]
    k2 = np.arange(N2)[:, None, None]
    T = np.exp(-2j * np.pi * (n1 * k2) / D) * np.exp(-2j * np.pi * (k1 * n1) / N1)
    I64 = np.exp(2j * np.pi * np.arange(N1)[:, None] * np.arange(N1)[None, :] / N1)
    I128 = np.exp(2j * np.pi * np.arange(N2)[:, None] * np.arange(N2)[None, :] / N2)
    tw = np.exp(2j * np.pi * np.arange(N2)[None, :] * np.arange(N1)[:, None] / D)
    f32 = lambda a: np.ascontiguousarray(a, np.float32)
    return (f32(T.real), f32(T.imag), f32(I64.real), f32(I64.imag),
            f32(I128.real), f32(I128.imag), f32(tw.real), f32(tw.imag))


def _shard_fn(xs1, xs2, w1r, w1i, w2r, w2i, Tr, Ti, I64r, I64i, I128r, I128i, twr, twi):
    # xs_j: (b, N1, PAD, HW) f32 class-sorted input
    def stage12(xs, wr, wi):
        f32 = jnp.float32
        Ar = jnp.einsum("rik,bril->brkl", wr, xs, preferred_element_type=f32)
        Ai = jnp.einsum("rik,bril->brkl", wi, xs, preferred_element_type=f32)
        Pr = jnp.einsum("knm,bmkl->bnkl", Tr, Ar) - jnp.einsum("knm,bmkl->bnkl", Ti, Ai)
        Pi = jnp.einsum("knm,bmkl->bnkl", Tr, Ai) + jnp.einsum("knm,bmkl->bnkl", Ti, Ar)
        return Pr, Pi

    P1r, P1i = stage12(xs1, w1r, w1i)
    P2r, P2i = stage12(xs2, w2r, w2i)
    Zr = jnp.einsum("bnkl,bnkl->bnk", P1r, P2r) - jnp.einsum("bnkl,bnkl->bnk", P1i, P2i)
    Zi = jnp.einsum("bnkl,bnkl->bnk", P1r, P2i) + jnp.einsum("bnkl,bnkl->bnk", P1i, P2r)
    Br = jnp.einsum("tn,bnk->btk", I64r, Zr) - jnp.einsum("tn,bnk->btk", I64i, Zi)
    Bi = jnp.einsum("tn,bnk->btk", I64r, Zi) + jnp.einsum("tn,bnk->btk", I64i, Zr)
    Cr = Br * twr - Bi * twi
    Ci = Br * twi + Bi * twr
    M = jnp.einsum("bmk,kt->bmt", Cr, I128r) - jnp.einsum("bmk,kt->bmt", Ci, I128i)
    cbp = jnp.transpose(M, (0, 2, 1)).reshape(M.shape[0], D) / D
    cbp = jnp.sign(cbp) * jnp.sqrt(jnp.abs(cbp) + 1e-5)
    nrm = jnp.maximum(
        jnp.sqrt(jnp.sum(cbp * cbp, axis=1, keepdims=True)), 1e-12
    )
    return cbp / nrm


_PMAP = None


def kernel(x, h1, h2, s1, s2):
    global _PMAP
    xf = np.asarray(x, np.float32).reshape(B, C, HW)
    perm1, w1r, w1i = _build_tables(h1, s1)
    perm2, w2r, w2i = _build_tables(h2, s2)
    xs1 = xf[:, perm1.ravel(), :].reshape(B, N1, PAD, HW).astype(BF16)
    xs2 = xf[:, perm2.ravel(), :].reshape(B, N1, PAD, HW).astype(BF16)
    bpc = B // NSHARDS
    xs1 = xs1.reshape(NSHARDS, bpc, N1, PAD, HW)
    xs2 = xs2.reshape(NSHARDS, bpc, N1, PAD, HW)
    if _PMAP is None:
        _PMAP = jax.pmap(
            _shard_fn, in_axes=(0, 0) + (None,) * 12, devices=jax.devices()[:NSHARDS]
        )
    out = _PMAP(xs1, xs2, w1r, w1i, w2r, w2i, *_consts())
    return np.asarray(out).reshape(B, D).astype(np.float32)
